# revision 1
# baseline (speedup 1.0000x reference)
"""Trainium2 Bass kernel for nn_Linear_48335561949661.

y = x @ dequant(weight, scale)^T
  x:      [4, 8, 7168] fp32
  weight: [18432, 7168] fp32 (block-dequantized by scale over 128x128 blocks)
  scale:  [144, 56] fp32
  y:      [4, 8, 18432] fp32

Sharding: column-parallel linear — weight/scale sharded along out_features
across 8 cores, x replicated, outputs concatenated on host.

Per-core device pipeline (memory-bound target, ~66MB weight stream/core):
  1. SWDGE cast-DMA weight fp32(HBM) -> fp16(SBUF), natural [o,i] layout
  2. PE transpose 128x128 blocks (fp16, 1 cyc/row) into PSUM
  3. DVE fused dequant-scale multiply + fp32->fp16 cast, PSUM -> SBUF
  4. PE matmul: x^T tiles stationary [128,32], w^T strips moving [128,512],
     fp32 accumulation in PSUM over the 56 K-tiles
  5. DVE evict y, DMA out fp32
"""

import sys

sys.path.insert(0, "/opt/trn_rl_repo")

import numpy as np

import concourse.bass as bass
import concourse.tile as tile
from concourse import bacc, mybir

FP32 = mybir.dt.float32
FP16 = mybir.dt.float16

BLOCK = 128  # dequant block size

# Full-problem constants (hardcoded per contract; kernel.py reads no files)
B, S, I, O = 4, 8, 7168, 18432
NCORES = 8
T = B * S                # 32 tokens
OSH = O // NCORES        # 2304 out rows per core


def build_nc(osh=OSH, i_feat=I, t=T, o_group=512, i_chunk=3584, debug=False,
             scale_mode="prescale_split", nwt=6, nwn=16, warm_mm=False,
             pt_bufs=3):
    """Build the per-core Bass program (SPMD: same program, 8 data shards).

    scale_mode: "fused" = one tensor_mul per PSUM bank with a step-0
    repeat AP on the scale operand; "split" = one tensor_scalar_mul per
    128-col sub-block (fallback if step-0 APs are rejected).
    """
    assert osh % BLOCK == 0 and i_feat % BLOCK == 0
    assert i_chunk % BLOCK == 0 and i_feat % i_chunk == 0
    n_ob = osh // BLOCK          # o-blocks per core (18)
    n_ib = i_feat // BLOCK       # i-blocks (56)
    n_ic = i_feat // i_chunk     # i chunks (2)
    ib_per_chunk = i_chunk // BLOCK

    # o-groups: up to o_group wide (multiple of 128)
    groups = []
    o0 = 0
    while o0 < osh:
        w = min(o_group, osh - o0)
        groups.append((o0, w))
        o0 += w

    nc = bacc.Bacc("TRN2", target_bir_lowering=False, debug=debug)

    w_d = nc.dram_tensor("w", [osh, i_feat], FP32, kind="ExternalInput")
    # xt packed on host: xt[p, b*t + tok] = x[tok, b*128 + p]
    xt_d = nc.dram_tensor("xt", [BLOCK, n_ib * t], FP16, kind="ExternalInput")
    # s packed on host: s[p, ib*n_ob + ob] = scale[ob, ib] (bcast over p)
    s_d = nc.dram_tensor("s", [BLOCK, n_ib * n_ob], FP32, kind="ExternalInput")
    id_d = nc.dram_tensor("ident", [BLOCK, BLOCK], FP16, kind="ExternalInput")
    y_d = nc.dram_tensor("y", [t, osh], FP32, kind="ExternalOutput")

    with tile.TileContext(nc) as tc:
        NWT = nwt  # wt ring slots
        NWN = nwn  # w natural-layout ring slots
        with (
            tc.tile_pool(name="const", bufs=1) as const_pool,
            tc.tile_pool(name="psum_t", bufs=pt_bufs, space="PSUM") as psum_t_pool,
            tc.tile_pool(name="psum_y", bufs=2, space="PSUM") as psum_y_pool,
            tc.tile_pool(name="psum_scr", bufs=1, space="PSUM") as psum_scr_pool,
        ):
            xt_sb = const_pool.tile([BLOCK, n_ib * t], FP16, tag="xt")
            s_sb = const_pool.tile([BLOCK, n_ib * n_ob], FP32, tag="s")
            id_sb = const_pool.tile([BLOCK, BLOCK], FP16, tag="id")
            # manually-rotated ring buffers: sub-range deps avoid the
            # pool slot-allocation waits that overflow the DVE wait slot
            o_gmax = max(w for _, w in groups)
            wt_ring = const_pool.tile([BLOCK, o_gmax * NWT], FP16, tag="wtr")
            wn_ring = const_pool.tile([BLOCK, i_chunk * NWN], FP16, tag="wnr")
            y_sb = const_pool.tile([t, osh], FP32, tag="ysb")
            scr_sb = const_pool.tile([BLOCK, 1], FP32, tag="scr")
            nc.sync.dma_start(xt_sb[:], xt_d.ap())
            nc.sync.dma_start(s_sb[:], s_d.ap())
            nc.sync.dma_start(id_sb[:], id_d.ap())
            # warmup absorbers: first consumer of each const per engine
            # carries the DMA wait, so steady-state ops keep a single
            # sync wait (the DVE/PE instruction structs encode only one).
            scr_pt = psum_scr_pool.tile([BLOCK, BLOCK], FP16, tag="scrt")
            nc.tensor.transpose(scr_pt[:], id_sb[:], id_sb[:])
            scr_py = psum_scr_pool.tile([t, BLOCK], FP32, tag="scrm")
            nc.tensor.matmul(scr_py[:], xt_sb[:, 0:t], id_sb[:],
                             start=True, stop=True)
            nc.vector.tensor_copy(scr_sb[:], s_sb[:, 0:1])

            ev_names = set()   # all eviction insts (same-engine WAW removal)
            wdma_names = set()  # all w-load DMAs (same-queue WAW removal)
            wdma_count = 0
            for (og0, ow) in groups:
                nob = ow // BLOCK
                py = psum_y_pool.tile([t, ow], FP32, tag="py")
                for ic in range(n_ic):
                    wn = []
                    for j in range(nob):
                        ob = og0 // BLOCK + j
                        slot = wdma_count % NWN
                        wdma_count += 1
                        wtile = wn_ring[:, slot * i_chunk:(slot + 1) * i_chunk]
                        dma = nc.gpsimd.dma_start(
                            wtile,
                            w_d.ap()[ob * BLOCK:(ob + 1) * BLOCK,
                                     ic * i_chunk:(ic + 1) * i_chunk],
                        )
                        # drop DMA->DMA WAW vs the slot's previous fill:
                        # same SWDGE queue + fixed engine<->partition map
                        # means per-ring FIFO already orders the writes
                        for d in list(dma.ins.sync_dependency_names()):
                            if d in wdma_names:
                                dma.ins.try_remove_dependency(d)
                        wdma_names.add(dma.ins.name)
                        if scale_mode == "prescale_split":
                            # dequant scale applied in place on the natural
                            # [o, i] tile: value varies per 128-wide i-block
                            s_ap = s_sb[:]
                            srep = bass.AP(
                                s_ap.tensor,
                                s_ap.offset + ic * ib_per_chunk * n_ob + ob,
                                [list(s_ap.ap[0]), [n_ob, ib_per_chunk],
                                 [0, BLOCK]],
                            )
                            psc = nc.vector.tensor_mul(wtile, wtile, srep)
                            for d in list(psc.ins.sync_dependency_names()):
                                if d in ev_names:
                                    psc.ins.try_remove_dependency(d)
                            ev_names.add(psc.ins.name)
                        wn.append(wtile)
                    for bb in range(ib_per_chunk):
                        ib = ic * ib_per_chunk + bb
                        pt = psum_t_pool.tile([BLOCK, ow],
                                              FP32 if warm_mm else FP16,
                                              tag="pt")
                        for j in range(nob):
                            src = wn[j][:, bb * BLOCK:(bb + 1) * BLOCK]
                            dst = pt[:, j * BLOCK:(j + 1) * BLOCK]
                            if warm_mm:
                                # normal-path matmul vs identity: same
                                # result as transpose-mode but counts as
                                # PE activity for the HAM clock gate
                                nc.tensor.matmul(dst, src, id_sb[:],
                                                 start=True, stop=True)
                            else:
                                nc.tensor.transpose(dst, src, id_sb[:])
                        wslot = ib % NWT
                        wt = wt_ring[:, wslot * o_gmax:wslot * o_gmax + ow]
                        scol = ib * n_ob + og0 // BLOCK
                        if scale_mode == "prescale_split":
                            # plain eviction, alternating DVE / ACT to
                            # split the PSUM-read-bound byte stream
                            if ib % 2 == 0:
                                ev = nc.vector.tensor_copy(wt, pt[:])
                                for d in list(
                                        ev.ins.sync_dependency_names()):
                                    if d in ev_names:
                                        ev.ins.try_remove_dependency(d)
                                ev_names.add(ev.ins.name)
                            else:
                                nc.scalar.activation(
                                    wt, pt[:],
                                    mybir.ActivationFunctionType.Copy)
                        elif scale_mode == "fused":
                            # scale operand: nob values, each repeated 128x
                            # along free dim, bcast layout already has all
                            # partitions equal.
                            s_ap = s_sb[:]
                            s_rep = bass.AP(
                                s_ap.tensor,
                                s_ap.offset + scol,
                                [list(s_ap.ap[0]), [1, nob], [0, BLOCK]],
                            )
                            ev = nc.vector.tensor_mul(wt, pt[:], s_rep)
                            # drop same-engine WAW edges vs previous ring
                            # occupants (DVE is in-order; the edge is
                            # implied) so the wait count stays within the
                            # DVE instruction's single sync-wait slot
                            for d in list(ev.ins.sync_dependency_names()):
                                if d in ev_names:
                                    ev.ins.try_remove_dependency(d)
                            ev_names.add(ev.ins.name)
                        else:
                            for j in range(nob):
                                nc.vector.tensor_scalar_mul(
                                    wt[:, j * BLOCK:(j + 1) * BLOCK],
                                    pt[:, j * BLOCK:(j + 1) * BLOCK],
                                    s_sb[:, scol + j:scol + j + 1],
                                )
                        for h0 in range(0, ow, 512):
                            hw_ = min(512, ow - h0)
                            nc.tensor.matmul(
                                py[:, h0:h0 + hw_],
                                xt_sb[:, ib * t:(ib + 1) * t],
                                wt[:, h0:h0 + hw_],
                                start=(ib == 0),
                                stop=(ib == n_ib - 1),
                            )
                yo = y_sb[:, og0:og0 + ow]
                nc.vector.tensor_copy(yo, py[:])
                nc.sync.dma_start(y_d.ap()[:, og0:og0 + ow], yo)

    nc.compile()
    return nc


def _legalize_waits(nc):
    """The TRN2 ISA structs encode a single sync wait; walrus rejects
    instructions with more. Drop waits that are implied by queue FIFO:
    SWDGE DMA->DMA same-queue writes are ordered by the descriptor ring,
    so a w-load DMA's DMASW lane wait (slot WAW / lane backpressure) is
    redundant once its cross-engine WAR wait is kept."""
    import bass_rust

    seq_ok = {"InstDrain", "InstEventSemaphore", "InstNoOp", "InstISA",
              "InstCall", "InstUnconditionalBranch", "InstRegisterMove"}
    for fn in nc.m.functions:
        for bb in fn.blocks:
            for ins in bb.instructions:
                nm = type(ins).__name__
                si = ins.sync_info
                if not si or len(si.on_wait) <= 1 or nm in seq_ok:
                    continue
                waits = list(si.on_wait)
                if nm == "InstDMACopy":
                    keep = [w for w in waits
                            if not w.ant_name.startswith("DMASW")]
                    if len(keep) <= 1:
                        ins.sync_info = bass_rust.SyncInfo(
                            on_wait=keep, on_update=list(si.on_update))
                        continue
                raise RuntimeError(
                    f"unlegalizable multi-wait {nm} {ins.name}: "
                    f"{[w.ant_name for w in waits]}")


def _pack_inputs(x, weight, scale, osh=OSH, i_feat=I, t=T, ncores=NCORES):
    """Host-side shard + repack. Returns per-core input maps."""
    n_ib = i_feat // BLOCK
    n_ob = osh // BLOCK
    xf = np.asarray(x, dtype=np.float32).reshape(t, i_feat)
    # [i, t] -> tiles [128, n_ib*t] with xt[p, b*t+tok] = xf[tok, b*128+p]
    xt = np.ascontiguousarray(
        xf.T.reshape(n_ib, BLOCK, t).transpose(1, 0, 2).reshape(BLOCK, n_ib * t)
    ).astype(np.float16)
    ident = np.eye(BLOCK, dtype=np.float16)
    in_maps = []
    for c in range(ncores):
        wsh = np.ascontiguousarray(weight[c * osh:(c + 1) * osh]).astype(
            np.float32, copy=False)
        ssh = np.asarray(scale[c * n_ob:(c + 1) * n_ob], dtype=np.float32)
        # s[p, ib*n_ob + ob] = ssh[ob, ib]
        spk = np.ascontiguousarray(
            np.broadcast_to(ssh.T.reshape(1, n_ib * n_ob), (BLOCK, n_ib * n_ob))
        ).astype(np.float32)
        in_maps.append({"w": wsh, "xt": xt, "s": spk, "ident": ident})
    return in_maps


_NC_CACHE = {}


def _get_nc(**kw):
    key = tuple(sorted(kw.items()))
    if key not in _NC_CACHE:
        _NC_CACHE[key] = build_nc(**kw)
    return _NC_CACHE[key]


def _run(x, weight, scale, trace=False, **trace_kw):
    from concourse.bass_utils import run_bass_kernel_spmd

    nc = _get_nc()
    in_maps = _pack_inputs(x, weight, scale)
    res = run_bass_kernel_spmd(
        nc, in_maps, core_ids=list(range(NCORES)), trace=trace, **trace_kw)
    y = np.concatenate([res.results[c]["y"] for c in range(NCORES)], axis=1)
    return np.ascontiguousarray(y.reshape(B, S, O).astype(np.float32)), res


def kernel(x, weight, scale):
    return _run(x, weight, scale)[0]



# revision 3
# speedup vs baseline: 1.2057x; 1.2057x over previous
"""Trainium2 Bass kernel for nn_Linear_48335561949661.

y = x @ dequant(weight, scale)^T
  x:      [4, 8, 7168] fp32
  weight: [18432, 7168] fp32 (block-dequantized by scale over 128x128 blocks)
  scale:  [144, 56] fp32
  y:      [4, 8, 18432] fp32

Sharding: column-parallel linear - weight/scale sharded along out_features
across 8 cores, x replicated, outputs concatenated on host.

Key restructure vs the v1 kernel (which PE-transposed + DVE-dequanted the
full weight stream on chip and was consumer-bound with a ~70us tail after
the DMA finished):
  * weight shard is transposed on the HOST -> [i, o] layout, so strips DMA
    straight into matmul-ready [128(i), osh] tiles. No PE transpose pass,
    no PSUM round-trip, no eviction traffic.
  * the dequant scale factors out of the matmul per (ob, ib) 128x128 block:
      y[t, o] = sum_ib s[ob, ib] * (x_ib @ w_ib^T)
    so it is folded into the tiny x stationary tiles (one DVE op per
    i-block over [128, 576]) instead of multiplying the 66MB weight stream.
  * per i-block the weight tile is consumed by 5 wide matmuls (stationary
    = scaled x for 4 o-blocks, moving = [128, 512] weight strip), which
    accumulate y directly in 5 persistent PSUM banks over all 56 i-blocks.
    Cross (ob_a, ob_b) sub-blocks of the PSUM output are don't-care; the
    host extracts the diagonal 32-row bands.

Per-core budget at the ~345GB/s measured SWDGE rate: 66MB weight stream
= 191us DMA; PE ~2.4us and DVE ~0.6us per 3.4us strip -> DMA-bound with
a ~6us tail (PSUM evict + 1.2MB result DMA).
"""

import sys

sys.path.insert(0, "/opt/trn_rl_repo")

import numpy as np

import concourse.bass as bass
import concourse.tile as tile
from concourse import bacc, mybir

FP32 = mybir.dt.float32
FP16 = mybir.dt.float16

BLOCK = 128  # dequant block size

# Full-problem constants (hardcoded per contract; kernel.py reads no files)
B, S, I, O = 4, 8, 7168, 18432
NCORES = 8
T = B * S                # 32 tokens
OSH = O // NCORES        # 2304 out rows per core
N_IB = I // BLOCK        # 56 i-blocks
N_OB = OSH // BLOCK      # 18 o-blocks per core

# matmul grouping: 4 o-blocks (512 cols) per PSUM group, 5 groups
GROUPS = [(0, 512), (512, 512), (1024, 512), (1536, 512), (2048, 256)]


def build_nc(nw=16, nx=16, debug=False):
    """Per-core Bass program (SPMD: same program, 8 data shards)."""
    nc = bacc.Bacc("TRN2", target_bir_lowering=False, debug=debug)

    # host-transposed weight shard: wT[i, o] fp32
    w_d = nc.dram_tensor("w", [I, OSH], FP32, kind="ExternalInput")
    # xt packed on host: xt[p, ib*T + tok] = x[tok, ib*128 + p], fp16
    xt_d = nc.dram_tensor("xt", [BLOCK, N_IB * T], FP16, kind="ExternalInput")
    # s packed on host (bcast over p): s[p, ib*N_OB + ob] = scale[ob, ib]
    s_d = nc.dram_tensor("s", [BLOCK, N_IB * N_OB], FP32, kind="ExternalInput")
    # raw PSUM contents [128, 2304]; host extracts diagonal 32-row bands
    yf_d = nc.dram_tensor("yf", [BLOCK, OSH], FP32, kind="ExternalOutput")

    with tile.TileContext(nc) as tc:
        with (
            tc.tile_pool(name="const", bufs=1) as const_pool,
            tc.tile_pool(name="psum_y", bufs=1, space="PSUM") as psum_y_pool,
        ):
            xt_sb = const_pool.tile([BLOCK, N_IB * T], FP16, tag="xt")
            s_sb = const_pool.tile([BLOCK, N_IB * N_OB], FP32, tag="s")
            # manually-rotated rings (sub-range deps, as in v1)
            w_ring = const_pool.tile([BLOCK, nw * OSH], FP16, tag="wr")
            xs_ring = const_pool.tile([BLOCK, nx * N_OB * T], FP16, tag="xs")
            yf_sb = const_pool.tile([BLOCK, OSH], FP32, tag="yf")
            nc.sync.dma_start(xt_sb[:], xt_d.ap())
            nc.sync.dma_start(s_sb[:], s_d.ap())

            py = []
            for g, (o0, ow) in enumerate(GROUPS):
                mw = ow // BLOCK * T  # stationary cols = out partitions
                py.append(psum_y_pool.tile([mw, ow], FP32, tag=f"py{g}",
                                           name=f"py{g}"))

            for ib in range(N_IB):
                wslot = ib % nw
                w_tile = w_ring[:, wslot * OSH:(wslot + 1) * OSH]
                nc.gpsimd.dma_start(
                    w_tile, w_d.ap()[ib * BLOCK:(ib + 1) * BLOCK, :])

                xslot = ib % nx
                xs_tile = xs_ring[:, xslot * N_OB * T:(xslot + 1) * N_OB * T]
                # xs[p, ob*T + tok] = xt[p, ib*T + tok] * s[p, ib*N_OB + ob]
                x_ap = xt_sb[:]
                in1 = bass.AP(x_ap.tensor, x_ap.offset + ib * T,
                              [list(x_ap.ap[0]), [0, N_OB], [1, T]])
                s_ap = s_sb[:]
                in2 = bass.AP(s_ap.tensor, s_ap.offset + ib * N_OB,
                              [list(s_ap.ap[0]), [1, N_OB], [0, T]])
                nc.vector.tensor_mul(xs_tile, in1, in2)

                for g, (o0, ow) in enumerate(GROUPS):
                    mw = ow // BLOCK * T
                    nc.tensor.matmul(
                        py[g][:, :],
                        xs_tile[:, (o0 // BLOCK) * T:(o0 // BLOCK) * T + mw],
                        w_tile[:, o0:o0 + ow],
                        start=(ib == 0),
                        stop=(ib == N_IB - 1),
                    )

            # evict PSUM -> SBUF (same partition base; host picks diagonals)
            for g, (o0, ow) in enumerate(GROUPS):
                mw = ow // BLOCK * T
                ev = yf_sb[0:mw, o0:o0 + ow]
                if g % 2 == 0:
                    nc.vector.tensor_copy(ev, py[g][:, :])
                else:
                    nc.scalar.activation(
                        ev, py[g][:, :], mybir.ActivationFunctionType.Copy)
            nc.sync.dma_start(yf_d.ap(), yf_sb[:])

    nc.compile()
    return nc


def _pack_inputs(x, weight, scale):
    """Host-side shard + repack. Returns per-core input maps."""
    xf = np.asarray(x, dtype=np.float32).reshape(T, I)
    # xt[p, ib*T + tok] = xf[tok, ib*128 + p]
    xt = np.ascontiguousarray(
        xf.T.reshape(N_IB, BLOCK, T).transpose(1, 0, 2).reshape(BLOCK, N_IB * T)
    ).astype(np.float16)
    in_maps = []
    for c in range(NCORES):
        wsh = np.ascontiguousarray(
            weight[c * OSH:(c + 1) * OSH].T)  # [I, OSH] fp32
        ssh = np.asarray(scale[c * N_OB:(c + 1) * N_OB], dtype=np.float32)
        # s[p, ib*N_OB + ob] = ssh[ob, ib]
        spk = np.ascontiguousarray(
            np.broadcast_to(ssh.T.reshape(1, N_IB * N_OB),
                            (BLOCK, N_IB * N_OB))).astype(np.float32)
        in_maps.append({"w": wsh, "xt": xt, "s": spk})
    return in_maps


# host extraction: y[tok, col] = yf[a*32 + tok, col], a = (col % 512) // 128
_COLS = np.arange(OSH)
_ROWS = ((_COLS % 512) // BLOCK)[None, :] * T + np.arange(T)[:, None]


def _unpack_output(res):
    y = np.empty((T, O), dtype=np.float32)
    for c in range(NCORES):
        yf = res.results[c]["yf"]  # [128, 2304]
        y[:, c * OSH:(c + 1) * OSH] = yf[_ROWS, _COLS[None, :]]
    return np.ascontiguousarray(y.reshape(B, S, O))


_NC_CACHE = {}


def _get_nc(**kw):
    key = tuple(sorted(kw.items()))
    if key not in _NC_CACHE:
        _NC_CACHE[key] = build_nc(**kw)
    return _NC_CACHE[key]


def _run(x, weight, scale, trace=False, **trace_kw):
    from concourse.bass_utils import run_bass_kernel_spmd

    nc = _get_nc()
    in_maps = _pack_inputs(x, weight, scale)
    res = run_bass_kernel_spmd(
        nc, in_maps, core_ids=list(range(NCORES)), trace=trace, **trace_kw)
    return _unpack_output(res), res


def kernel(x, weight, scale):
    return _run(x, weight, scale)[0]


# revision 6
# speedup vs baseline: 1.4793x; 1.2269x over previous
"""Trainium2 Bass kernel for nn_Linear_48335561949661.

y = x @ dequant(weight, scale)^T
  x:      [4, 8, 7168] fp32
  weight: [18432, 7168] fp32 (block-dequantized by scale over 128x128 blocks)
  scale:  [144, 56] fp32
  y:      [4, 8, 18432] fp32

Sharding: column-parallel linear - weight/scale sharded along out_features
across 8 cores, x replicated, outputs concatenated on host.

Structure (v2): the weight shard is transposed on the HOST to [i, o] so
strips DMA straight into matmul-ready [128(i), osh] tiles, and the dequant
scale is folded into the tiny x stationary tiles (it factors per 128x128
block: y[t,o] = sum_ib s[ob,ib] * (x_ib @ w_ib^T)). The 66MB weight stream
flows HBM -> SBUF -> PE untouched; per i-block one DVE op builds the scaled
stationary and 5 wide matmuls accumulate y in 5 persistent PSUM banks.
Cross (ob_a, ob_b) sub-blocks of the PSUM tiles are don't-care; the host
extracts the diagonal 32-row bands.

dma modes:
  swdge16:  SWDGE cast-DMA fp32->fp16 (gpsimd queue), fp16 matmuls.
  hwdge32r: plain HWDGE fp32 loads (0.6us startup, RTL descriptor gen, no
            Q7 in the loop), float32r matmuls (1 cyc/row at moving>=256).
"""

import sys

sys.path.insert(0, "/opt/trn_rl_repo")

import numpy as np

import concourse.bass as bass
import concourse.tile as tile
from concourse import bacc, mybir

FP32 = mybir.dt.float32
FP32R = mybir.dt.float32r
FP16 = mybir.dt.float16

BLOCK = 128  # dequant block size

# Full-problem constants (hardcoded per contract; kernel.py reads no files)
B, S, I, O = 4, 8, 7168, 18432
NCORES = 8
T = B * S                # 32 tokens
OSH = O // NCORES        # 2304 out rows per core
N_IB = I // BLOCK        # 56 i-blocks
N_OB = OSH // BLOCK      # 18 o-blocks per core

# matmul grouping: 4 o-blocks (512 cols) per PSUM group, 5 groups
GROUPS = [(0, 512), (512, 512), (1024, 512), (1536, 512), (2048, 256)]

MODE = "swdge16"         # overridden via _get_nc kwargs
IPD = 2                  # i-blocks per weight DMA


def build_nc(mode=MODE, ipd=IPD, nw=16, nx=16, debug=False):
    """Per-core Bass program (SPMD: same program, 8 data shards).

    nw: ring depth in i-blocks (must be a multiple of ipd).
    """
    assert N_IB % ipd == 0 and nw % ipd == 0
    wdt = FP16 if mode == "swdge16" else FP32R
    xdt = FP16 if mode == "swdge16" else FP32
    xsdt = FP16 if mode == "swdge16" else FP32R
    nc = bacc.Bacc("TRN2", target_bir_lowering=False, debug=debug)

    # host-transposed weight shard: wT[i, o] fp32 (declared fp32r in hwdge
    # mode: the PE consumes the raw fp32 bits; HW-probed rel err 1.5e-4)
    w_d = nc.dram_tensor("w", [I, OSH], FP32 if mode == "swdge16" else FP32R,
                         kind="ExternalInput")
    # xt packed on host: xt[p, ib*T + tok] = x[tok, ib*128 + p]
    xt_d = nc.dram_tensor("xt", [BLOCK, N_IB * T], xdt, kind="ExternalInput")
    # s packed on host (bcast over p): s[p, ib*N_OB + ob] = scale[ob, ib]
    s_d = nc.dram_tensor("s", [BLOCK, N_IB * N_OB], FP32, kind="ExternalInput")
    # raw PSUM contents [128, 2304]; host extracts diagonal 32-row bands
    yf_d = nc.dram_tensor("yf", [BLOCK, OSH], FP32, kind="ExternalOutput")

    with tile.TileContext(nc) as tc:
        with (
            tc.tile_pool(name="const", bufs=1) as const_pool,
            tc.tile_pool(name="psum_y", bufs=1, space="PSUM") as psum_y_pool,
        ):
            xt_sb = const_pool.tile([BLOCK, N_IB * T], xdt, tag="xt")
            s_sb = const_pool.tile([BLOCK, N_IB * N_OB], FP32, tag="s")
            # manually-rotated rings (sub-range deps, as in v1)
            w_ring = const_pool.tile([BLOCK, nw * OSH], wdt, tag="wr")
            xs_ring = const_pool.tile([BLOCK, nx * N_OB * T], xsdt, tag="xs")
            yf_sb = const_pool.tile([BLOCK, OSH], FP32, tag="yf")
            nc.sync.dma_start(xt_sb[:], xt_d.ap())
            nc.sync.dma_start(s_sb[:], s_d.ap())

            py = []
            for g, (o0, ow) in enumerate(GROUPS):
                mw = ow // BLOCK * T  # stationary cols = out partitions
                py.append(psum_y_pool.tile([mw, ow], FP32, tag=f"py{g}",
                                           name=f"py{g}"))

            w_base = w_d.ap()
            for ib0 in range(0, N_IB, ipd):
                wslot = ib0 % nw
                big = w_ring[:, wslot * OSH:(wslot + ipd) * OSH]
                # DRAM AP: [part(row) 128][block ipd][col 2304]
                src = bass.AP(w_base.tensor,
                              w_base.offset + ib0 * BLOCK * OSH,
                              [[OSH, BLOCK], [BLOCK * OSH, ipd], [1, OSH]])
                if mode == "swdge16":
                    nc.gpsimd.dma_start(big, src)
                else:
                    nc.sync.dma_start(big, src)

                for ib in range(ib0, ib0 + ipd):
                    w_tile = w_ring[:, (ib % nw) * OSH:(ib % nw + 1) * OSH]
                    xslot = ib % nx
                    xs_tile = xs_ring[:, xslot * N_OB * T:
                                      (xslot + 1) * N_OB * T]
                    # xs[p, ob*T+tok] = xt[p, ib*T+tok] * s[p, ib*N_OB+ob]
                    x_ap = xt_sb[:]
                    in1 = bass.AP(x_ap.tensor, x_ap.offset + ib * T,
                                  [list(x_ap.ap[0]), [0, N_OB], [1, T]])
                    s_ap = s_sb[:]
                    in2 = bass.AP(s_ap.tensor, s_ap.offset + ib * N_OB,
                                  [list(s_ap.ap[0]), [1, N_OB], [0, T]])
                    nc.vector.tensor_mul(xs_tile, in1, in2)

                    for g, (o0, ow) in enumerate(GROUPS):
                        mw = ow // BLOCK * T
                        lhsT = xs_tile[:, (o0 // BLOCK) * T:
                                       (o0 // BLOCK) * T + mw]
                        rhs = w_tile[:, o0:o0 + ow]
                        nc.tensor.matmul(
                            py[g][:, :], lhsT, rhs,
                            start=(ib == 0), stop=(ib == N_IB - 1))

            # evict PSUM -> SBUF (same partition base; host picks diagonals)
            for g, (o0, ow) in enumerate(GROUPS):
                mw = ow // BLOCK * T
                ev = yf_sb[0:mw, o0:o0 + ow]
                if g % 2 == 0:
                    nc.vector.tensor_copy(ev, py[g][:, :])
                else:
                    nc.scalar.activation(
                        ev, py[g][:, :], mybir.ActivationFunctionType.Copy)
            nc.sync.dma_start(yf_d.ap(), yf_sb[:])

    nc.compile()
    return nc


def _pack_inputs(x, weight, scale, mode=MODE):
    """Host-side shard + repack. Returns per-core input maps."""
    xdt = np.float16 if mode == "swdge16" else np.float32
    xf = np.asarray(x, dtype=np.float32).reshape(T, I)
    # xt[p, ib*T + tok] = xf[tok, ib*128 + p]
    xt = np.ascontiguousarray(
        xf.T.reshape(N_IB, BLOCK, T).transpose(1, 0, 2).reshape(BLOCK, N_IB * T)
    ).astype(xdt)
    in_maps = []
    for c in range(NCORES):
        wsh = np.ascontiguousarray(
            weight[c * OSH:(c + 1) * OSH].T)  # [I, OSH] fp32
        ssh = np.asarray(scale[c * N_OB:(c + 1) * N_OB], dtype=np.float32)
        # s[p, ib*N_OB + ob] = ssh[ob, ib]
        spk = np.ascontiguousarray(
            np.broadcast_to(ssh.T.reshape(1, N_IB * N_OB),
                            (BLOCK, N_IB * N_OB))).astype(np.float32)
        in_maps.append({"w": wsh, "xt": xt, "s": spk})
    return in_maps


# host extraction: y[tok, col] = yf[a*32 + tok, col], a = (col % 512) // 128
_COLS = np.arange(OSH)
_ROWS = ((_COLS % 512) // BLOCK)[None, :] * T + np.arange(T)[:, None]


def _unpack_output(res):
    y = np.empty((T, O), dtype=np.float32)
    for c in range(NCORES):
        yf = res.results[c]["yf"]  # [128, 2304]
        y[:, c * OSH:(c + 1) * OSH] = yf[_ROWS, _COLS[None, :]]
    return np.ascontiguousarray(y.reshape(B, S, O))


_NC_CACHE = {}


def _get_nc(**kw):
    key = tuple(sorted(kw.items()))
    if key not in _NC_CACHE:
        _NC_CACHE[key] = build_nc(**kw)
    return _NC_CACHE[key]


def _run(x, weight, scale, trace=False, mode=MODE, ipd=IPD, **trace_kw):
    from concourse.bass_utils import run_bass_kernel_spmd

    nc = _get_nc(mode=mode, ipd=ipd)
    in_maps = _pack_inputs(x, weight, scale, mode=mode)
    res = run_bass_kernel_spmd(
        nc, in_maps, core_ids=list(range(NCORES)), trace=trace, **trace_kw)
    return _unpack_output(res), res


def kernel(x, weight, scale):
    return _run(x, weight, scale)[0]
